# revision 35
# baseline (speedup 1.0000x reference)
"""Single-head causal attention on 8 TRN2 NeuronCores (Bass/Tile).

Problem: x [4, 2048, 1024] fp32; wq/wk/wv [1024, 128]; wo [128, 1024].
out = softmax_causal((x@wq)(x@wk)^T / sqrt(128)) @ (x@wv) @ wo

Sharding: 8 cores = 4 batches x 2 query-interleavings (zebra pattern:
core parity p takes seq blocks {4g+o0, 4g+o1} of each group g of 4
blocks). Host permutes+transposes x so each core's query slots are
contiguous; slot j = permuted q cols [512j : 512j+256] = seq blocks
{4j+order[0], 4j+order[1]}, attending permuted key prefix [0 : 512(j+1)].

Schedule notes (all perf-load-bearing):
 - x arrives seq-half-major ([128d x 1024s] granules, half 0 on the
   Scalar HWDGE queue, half 1 on Sync behind the weights) so projections
   start with the first granule; the proj phase is DMA-paced end to end.
 - Projections per seq-half (QT/KT/VT accumulate d-chunk-outer in 5 psum
   banks); psum->SBUF copies split DVE (vt, qt) / ACT (kt).
 - Attention processes slot PAIRS ({0,1}, {2,3}): shared key groups
   stream 512-wide q so matmuls are N=512 and LDWEIGHTS hides under
   streaming. Score tiles [P,1024] (2 banks, bufs=2), ONE exp per tile
   (ACT per-op overhead). Causal masks are ADDITIVE (-30000), applied by
   PE matmul-accumulation of the mask through an identity lhsT - no
   vector op in the exp->den/AV chain. den/AV skip the mask regions that
   are provably all-zero for both parities (kb1/kb3 lo-halves of diag
   groups).
 - Every multi-consumer SBUF tensor is split per half/pair (qt/kt/vt/v/
   ctxt/den/PT-per-group): Tile tracks deps at whole-tile granularity,
   so shared slabs create false WAR edges that serialize phases.
 - Output projection: 1/den scaling fused into the mandatory psum->SBUF
   copies (3 DVE / 1 ACT per block); fp16 stores on Sync; den
   partition-ified by tiny gpsimd DMAs, reciprocal after ([1,N] DVE
   reciprocal is ~8ns/elem - never do that).
"""

import numpy as np

import concourse.bass as bass
from concourse import bacc
import concourse.mybir as mybir
import concourse.tile as tile
from concourse.bass_utils import run_bass_kernel_spmd
from concourse.masks import make_identity

F32 = mybir.dt.float32
F16 = mybir.dt.float16

D_MODEL = 1024
D_HEAD = 128
SEQ = 2048
BATCH = 4
NCORES = 8
P = 128            # partitions / block size
DC = D_MODEL // P  # 8 d_model chunks
NB = SEQ // P      # 16 seq blocks
NSLOT = 4
QW = 256           # queries per slot
NQ = NSLOT * QW    # 1024 queries per core
HS = SEQ // 2      # seq half
SCALE = 1.0 / float(np.sqrt(D_HEAD))
EXP_BIAS = -3.0    # exp(scale*s - 3): keeps exp outputs < 1e4 (fp16-safe)
MASK_NEG = -30000.0


def block_order(parity: int) -> list[int]:
    order = []
    for g in range(4):
        if parity == 0:
            order += [4 * g, 4 * g + 3, 4 * g + 1, 4 * g + 2]
        else:
            order += [4 * g + 1, 4 * g + 2, 4 * g, 4 * g + 3]
    return order


def make_mask01(parity: int) -> np.ndarray:
    """0/1 keep-mask for the diagonal 512-key group of a slot,
    transposed: [512 k, 256 q]."""
    P4 = block_order(parity)[:4]
    m = np.zeros((512, 256), dtype=np.float16)
    kr = np.arange(P)[:, None]
    qc = np.arange(P)[None, :]
    tri = (kr <= qc).astype(np.float16)
    for kb2 in range(4):
        K = P4[kb2]
        for qb2 in range(2):
            Q = P4[qb2]
            blk = m[P * kb2:P * (kb2 + 1), P * qb2:P * (qb2 + 1)]
            if K < Q:
                blk[:] = 1.0
            elif K > Q:
                blk[:] = 0.0
            else:
                blk[:] = tri
    return m


def _attention_kernel(tc: tile.TileContext, xt_d, wq_d, wk_d, wv_d, wo_d,
                      maskt_d, out_d):
    nc = tc.nc

    with (
        tc.tile_pool(name="const", bufs=1) as const_pool,
        tc.tile_pool(name="big", bufs=1) as big_pool,
        tc.tile_pool(name="ptp", bufs=4) as pt_pool,
        tc.tile_pool(name="outp", bufs=3) as out_pool,
    ):
        # ---- weights + mask on the Sync HWDGE ring ----
        wq_sb = const_pool.tile([P, DC, P], F16)
        nc.sync.dma_start(out=wq_sb, in_=wq_d.rearrange("p (c h) -> p c h", h=P))
        wk_sb = const_pool.tile([P, DC, P], F16)
        nc.sync.dma_start(out=wk_sb, in_=wk_d.rearrange("p (c h) -> p c h", h=P))
        wv_sb = const_pool.tile([P, DC, P], F16)
        nc.sync.dma_start(out=wv_sb, in_=wv_d.rearrange("p (c h) -> p c h", h=P))

        # ---- x seq-half-major: half 0 on Scalar HWDGE, half 1 on Sync ----
        xt_sb = [[None, None] for _ in range(DC)]
        for h in range(2):
            eng = nc.scalar if h == 0 else nc.sync
            for c in range(DC):
                t = big_pool.tile([P, HS], F16, name=f"xt_sb{c}_{h}")
                eng.dma_start(
                    out=t, in_=xt_d[P * c:P * (c + 1), HS * h:HS * (h + 1)])
                xt_sb[c][h] = t

        # mask + wo queue behind x on Sync: not needed until attention,
        # so they must not steal early DMA bandwidth from x half 0
        maskt_sb = const_pool.tile([P, 4, QW], F16)
        nc.sync.dma_start(out=maskt_sb,
                          in_=maskt_d.rearrange("p (b q) -> p b q", q=QW))
        wo_sb = const_pool.tile([P, D_MODEL], F16)
        nc.sync.dma_start(out=wo_sb, in_=wo_d)

        # ---- constants ----
        ident = const_pool.tile([P, P], F16)
        make_identity(nc, ident)
        ones = const_pool.tile([P, 1], F16)
        nc.vector.memset(ones, 1.0)
        expbias = const_pool.tile([P, 1], F32)
        nc.vector.memset(expbias, EXP_BIAS)
        onef32 = const_pool.tile([1, 1], F32)
        nc.vector.memset(onef32, 1.0)
        # dummy exp: pulls the ACT exp table load to t=0
        actwarm = const_pool.tile([P, 1], F32)
        nc.scalar.activation(out=actwarm, in_=expbias,
                             func=mybir.ActivationFunctionType.Exp)

        # per-quarter / per-pair SBUF tensors. Split fine: Tile tracks
        # deps at whole-tile granularity, and a tile with two writer
        # engines (DVE+ACT copies) makes every reader wait on both.
        qt_h = [big_pool.tile([P, 512], F16, name=f"qt{h}") for h in range(2)]
        kt_q = [big_pool.tile([P, 512], F16, name=f"kt{q}") for q in range(4)]
        vt_q = [big_pool.tile([P, 512], F16, name=f"vt{q}") for q in range(4)]
        v_h = [big_pool.tile([P, HS], F16, name=f"v{h}") for h in range(2)]
        ctxt_p = [big_pool.tile([P, 512], F16, name=f"ctxt{a}") for a in range(2)]
        den_p = [big_pool.tile([1, 512], F32, name=f"den{a}") for a in range(2)]
        rden_p = [big_pool.tile([P, 4], F32, name=f"rden{a}") for a in range(2)]

        def kt_blk(kb):
            return kt_q[kb // 4][:, P * (kb % 4):P * (kb % 4 + 1)]

        def v_blk(kb):
            return v_h[kb // 8][:, P * (kb % 8):P * (kb % 8 + 1)]

        def proj_half(h, pj):
            """QT/KT/VT for seq half h, d-chunk outer; copies to SBUF."""
            qt_ps = pj.tile([P, 512], F32, name=f"qt_ps{h}")
            kt_ps = [pj.tile([P, 512], F32, name=f"kt_ps{h}_{i}")
                     for i in range(2)]
            vt_ps = [pj.tile([P, 512], F32, name=f"vt_ps{h}_{i}")
                     for i in range(2)]
            for c in range(DC):
                xh = xt_sb[c][h]
                xr = xh.rearrange("p (g q) -> p g q", q=QW)
                st, sp = (c == 0), (c == DC - 1)
                nc.tensor.matmul(qt_ps, lhsT=wq_sb[:, c, :],
                                 rhs=xr[:, 0:3:2, :], start=st, stop=sp,
                                 skip_group_check=True)
                for i in range(2):
                    nc.tensor.matmul(kt_ps[i], lhsT=wk_sb[:, c, :],
                                     rhs=xh[:, 512 * i:512 * (i + 1)],
                                     start=st, stop=sp, skip_group_check=True)
                for i in range(2):
                    nc.tensor.matmul(vt_ps[i], lhsT=wv_sb[:, c, :],
                                     rhs=xh[:, 512 * i:512 * (i + 1)],
                                     start=st, stop=sp, skip_group_check=True)
            # copies (vt first: transposes wait on them). For half 0,
            # balance DVE/ACT; for half 1 keep ACT free — attA's exps
            # run concurrently with these copies.
            if h == 0:
                nc.vector.tensor_copy(vt_q[0], vt_ps[0])
                nc.scalar.copy(vt_q[1], vt_ps[1])
                nc.vector.tensor_copy(kt_q[0], kt_ps[0])
                nc.scalar.copy(kt_q[1], kt_ps[1])
                nc.vector.tensor_copy(qt_h[h], qt_ps)
            else:
                nc.vector.tensor_copy(vt_q[2], vt_ps[0])
                nc.vector.tensor_copy(vt_q[3], vt_ps[1])
                nc.vector.tensor_copy(kt_q[2], kt_ps[0])
                nc.vector.tensor_copy(kt_q[3], kt_ps[1])
                nc.vector.tensor_copy(qt_h[h], qt_ps)

        def transposes(h, trp):
            for b in range(8):
                ptr = trp.tile([P, P], F16, tag="tr", bufs=2,
                               name=f"ptr{h}_{b}")
                nc.tensor.transpose(
                    ptr, vt_q[2 * h + b // 4][:, P * (b % 4):P * (b % 4 + 1)],
                    ident)
                nc.vector.tensor_copy(v_h[h][:, P * b:P * (b + 1)], ptr)

        def attention_pair(a, att):
            """Pair a: slots {2a, 2a+1}; pair q cols 0..511 = qt halves
            a (lo slot, cols 0..255) and hi slot (256..511)... pair q =
            [slot 2a | slot 2a+1] = qt_h[a] fully. Key groups 0..2a-1
            full, group 2a diag-masked on slot-lo, group 2a+1 slot-hi
            only."""
            jlo, jhi = 2 * a, 2 * a + 1
            nfull = jhi            # groups 0..jhi-1 stream 512q
            qt_pair = qt_h[a]
            qt_hi = qt_h[a][:, 256:512]

            den_ps = att.tile([1, 512], F32, tag="den", bufs=1,
                              name=f"den_ps{a}")
            ctx_ps = att.tile([P, 512], F32, tag="ctx", bufs=1,
                              name=f"ctx_ps{a}")
            pt = {}

            def st_exp(g):
                """Scores (+additive mask on diag regions) + exp for key
                group g -> its own PT tile. PT layouts: wide groups
                [4kb x 512q]; narrow [4kb x 256q]."""
                wide = g < nfull or g == jlo
                qw = 512 if wide else 256
                qtr = qt_pair if wide else qt_hi
                ptile = pt_pool.tile([P, 2048], F16, tag="pt",
                                     name=f"pt{a}_{g}")
                pt[g] = ptile
                diag = (g == jlo) if wide else True
                nhalf = 2 if wide else 1
                for half in range(nhalf):
                    stp = att.tile([P, 1024], F32, tag="st", bufs=2,
                                   name=f"st{a}_{g}_{half}")
                    for k2h in range(4 // nhalf):
                        k2 = half * 2 + k2h if wide else k2h
                        kb = 4 * g + k2
                        base = qw * k2h  # kb's col base in stp
                        nc.tensor.matmul(stp[:, base:base + qw],
                                         lhsT=kt_blk(kb), rhs=qtr,
                                         start=True, stop=not diag,
                                         skip_group_check=True)
                        if not diag:
                            continue
                        # additive causal mask (identity-matmul accum) on
                        # the diag slot's 256 q (= first 256 cols of the
                        # kb when wide). kb1/kb3: q0 sub-block is all
                        # -inf for both parities -> den/av skip it, so
                        # only the q1 half needs the mask rows.
                        if k2 in (0, 2):
                            nc.tensor.matmul(
                                stp[:, base:base + 256],
                                lhsT=ident, rhs=maskt_sb[:, k2, :],
                                start=False, stop=True,
                                skip_group_check=True)
                        else:
                            nc.tensor.matmul(
                                stp[:, base + P:base + 256],
                                lhsT=ident, rhs=maskt_sb[:, k2, P:QW],
                                start=False, stop=True,
                                skip_group_check=True)
                    off = 1024 * half
                    nc.scalar.activation(
                        out=ptile[:, off:off + 1024], in_=stp,
                        func=mybir.ActivationFunctionType.Exp,
                        bias=expbias, scale=SCALE)

            ngroups = jhi + 1
            ndenav = 0

            def denav(g):
                """den+AV for key group g. Skips regions that the causal
                mask provably zeroes for both parities: diag-group kb1/
                kb3 slot-lo halves."""
                nonlocal ndenav
                wide = g < nfull or g == jlo
                ptile = pt[g]
                first = (g == 0)
                last = (ndenav == ngroups - 1)
                parts = []  # (kb, pt_off, den_off, width)
                for k2 in range(4):
                    kb = 4 * g + k2
                    if wide:
                        base = 512 * k2
                        if g == jlo and k2 in (1, 3):
                            # skip slot-lo q0 sub-block (all-masked)
                            parts.append((kb, base + P, P, 512 - P))
                        else:
                            parts.append((kb, base, 0, 512))
                    else:
                        base = 256 * k2
                        if k2 in (1, 3):
                            parts.append((kb, base + P, 256 + P, P))
                        else:
                            parts.append((kb, base, 256, 256))
                for i, (kb, po, do, w) in enumerate(parts):
                    nc.tensor.matmul(den_ps[:, do:do + w], lhsT=ones,
                                     rhs=ptile[:, po:po + w],
                                     start=(first and i == 0),
                                     stop=(last and i == len(parts) - 1),
                                     skip_group_check=True)
                for i, (kb, po, do, w) in enumerate(parts):
                    nc.tensor.matmul(ctx_ps[:, do:do + w], lhsT=v_blk(kb),
                                     rhs=ptile[:, po:po + w],
                                     start=(first and i == 0),
                                     stop=(last and i == len(parts) - 1),
                                     skip_group_check=True)
                ndenav += 1

            # interleave: scores run one group ahead of den/av
            st_exp(0)
            for g in range(1, ngroups):
                st_exp(g)
                denav(g - 1)
            denav(ngroups - 1)

            # finish: ctx/den to SBUF; den partition-ified by PE
            # transpose-matmuls (lhsT = [1,128] den row, rhs = ident[0,0]
            # -> [128,1] column) into a psum tile sharing den's bank;
            # then ONE ACT Reciprocal reads psum and writes rden SBUF.
            # (The earlier gpsimd-DMA broadcast put ~2us of SWDGE latency
            # on the last pair's critical path.)
            nc.vector.tensor_copy(ctxt_p[a], ctx_ps)
            nc.vector.tensor_copy(den_p[a], den_ps)
            dent_ps = att.tile([P, 4], F32, tag="den", bufs=1,
                               name=f"dent_ps{a}")
            for qb2 in range(4):
                nc.tensor.matmul(dent_ps[:, qb2:qb2 + 1],
                                 lhsT=den_p[a][0:1, P * qb2:P * (qb2 + 1)],
                                 rhs=onef32,
                                 start=(qb2 == 0), stop=(qb2 == 3),
                                 skip_group_check=True)
            nc.vector.reciprocal(rden_p[a], dent_ps)

        def out_block(qb, att, act_heavy=False, split_store=False,
                      op_bufs=2):
            """Output projection for 128-q block qb (qt col block). 1/den
            scaling fuses into the mandatory psum->SBUF copies. While ACT
            is exp-bound (pair-A blocks) it takes 1 of 4 halves; after
            the last exp (pair-B blocks) the split is even (act_heavy).
            split_store issues each half's store as soon as it's scaled
            (for the last block, whose store is the kernel tail)."""
            a, qb2 = qb // 4, qb % 4
            rd = rden_p[a][:, qb2:qb2 + 1]
            ctxb = ctxt_p[a][:, P * qb2:P * (qb2 + 1)]
            ot = out_pool.tile([P, D_MODEL], F16, tag="ot")
            for t in range(2):
                ps = att.tile([P, 512], F32, tag="op", bufs=op_bufs,
                              name=f"op{qb}_{t}")
                nc.tensor.matmul(ps, lhsT=ctxb,
                                 rhs=wo_sb[:, 512 * t:512 * (t + 1)],
                                 start=True, stop=True,
                                 skip_group_check=True)
                on_act = (t == 1) if act_heavy else (t == 1 and qb % 2 == 0)
                if on_act:
                    nc.scalar.mul(ot[:, 512 * t:512 * (t + 1)], ps, rd)
                else:
                    nc.vector.tensor_scalar_mul(
                        ot[:, 512 * t:512 * (t + 1)], ps, rd)
                if split_store:
                    nc.sync.dma_start(
                        out=out_d[P * qb:P * (qb + 1),
                                  512 * t:512 * (t + 1)],
                        in_=ot[:, 512 * t:512 * (t + 1)])
            if not split_store:
                nc.sync.dma_start(out=out_d[P * qb:P * (qb + 1), :], in_=ot)

        # ---------------- schedule ----------------
        # PE warm-up: the HAM clock gate needs ~3.4us of sustained PE
        # activity to lift the clock from 1.2 to 2.4 GHz, and the first
        # ~4us of the kernel is DMA-wait. Burn it on dummy matmuls so the
        # projections run warm (measured ~6us of cold-clock otherwise).
        with tc.tile_pool(name="warm", bufs=1, space="PSUM") as warm:
            wps = warm.tile([P, P], F32, name="warm_ps")
            for _ in range(34):
                nc.tensor.matmul(wps, lhsT=ident, rhs=ident,
                                 start=True, stop=True,
                                 skip_group_check=True)
        # Both projections back to back (the whole proj phase is paced by
        # the single x DMA stream; pjB's matmuls fill pjA's arrival gaps).
        # PSUM budget forces the sequencing: pjA(5) -> trA(2)+pjB(5)=7 ->
        # trB(2)+attA(6)=8 -> attB(8).
        with tc.tile_pool(name="pjA", bufs=1, space="PSUM") as pjA:
            proj_half(0, pjA)
        with tc.tile_pool(name="trA", bufs=1, space="PSUM") as trA:
            transposes(0, trA)
            with tc.tile_pool(name="pjB", bufs=1, space="PSUM") as pjB:
                proj_half(1, pjB)
        with tc.tile_pool(name="trB", bufs=1, space="PSUM") as trB:
            transposes(1, trB)
            with tc.tile_pool(name="attA", bufs=1, space="PSUM") as attA:
                attention_pair(0, attA)
        with tc.tile_pool(name="attB", bufs=1, space="PSUM") as attB:
            # pair-A output blocks first: their deps (ctxt/rden) are done
            # by now, so they fill the attA->attB seam.
            for qb in range(4):
                out_block(qb, attB)
            attention_pair(1, attB)
        with tc.tile_pool(name="attB2", bufs=1, space="PSUM") as attB2:
            # pair-B outs in their own pool: the score banks are dead, so
            # op gets bufs=4 and the scale copies pipeline fully.
            for qb in range(4, 8):
                out_block(qb, attB2, act_heavy=True, split_store=(qb == 7),
                          op_bufs=4)


_NC_CACHE = None


def build_nc() -> bass.Bass:
    global _NC_CACHE
    if _NC_CACHE is not None:
        return _NC_CACHE
    nc = bacc.Bacc("TRN2", target_bir_lowering=False, debug=False)
    xt_d = nc.dram_tensor("xt", [D_MODEL, SEQ], F16, kind="ExternalInput").ap()
    wq_d = nc.dram_tensor("wq", [P, DC * D_HEAD], F16, kind="ExternalInput").ap()
    wk_d = nc.dram_tensor("wk", [P, DC * D_HEAD], F16, kind="ExternalInput").ap()
    wv_d = nc.dram_tensor("wv", [P, DC * D_HEAD], F16, kind="ExternalInput").ap()
    wo_d = nc.dram_tensor("wo", [D_HEAD, D_MODEL], F16, kind="ExternalInput").ap()
    maskt_d = nc.dram_tensor("maskt", [P, 4 * QW], F16, kind="ExternalInput").ap()
    out_d = nc.dram_tensor("out", [NQ, D_MODEL], F16, kind="ExternalOutput").ap()
    with tile.TileContext(nc) as tc:
        _attention_kernel(tc, xt_d, wq_d, wk_d, wv_d, wo_d, maskt_d, out_d)
    nc.compile()
    _NC_CACHE = nc
    return nc


def _chunk_major(w):
    """[1024, 128] -> [128, 8*128]: row p holds chunks c of w[128c+p, :]."""
    return np.ascontiguousarray(
        w.reshape(DC, P, D_HEAD).transpose(1, 0, 2).reshape(P, DC * D_HEAD))


def kernel(x, wq, wk, wv, wo, _trace=False, _trace_kwargs=None):
    x = np.asarray(x, dtype=np.float32)
    wq_h = _chunk_major(np.asarray(wq, dtype=np.float32).astype(np.float16))
    wk_h = _chunk_major(np.asarray(wk, dtype=np.float32).astype(np.float16))
    wv_h = _chunk_major(np.asarray(wv, dtype=np.float32).astype(np.float16))
    wo_h = np.ascontiguousarray(np.asarray(wo, dtype=np.float32).astype(np.float16))

    nc = build_nc()

    masks = {}
    for p in (0, 1):
        m = (1.0 - make_mask01(p).astype(np.float32)) * MASK_NEG  # additive
        m = m.astype(np.float16)  # [512 k, 256 q]
        masks[p] = np.ascontiguousarray(
            m.reshape(4, P, QW).transpose(1, 0, 2).reshape(P, 4 * QW))
    in_maps = []
    for core in range(NCORES):
        b, parity = core // 2, core % 2
        order = block_order(parity)
        perm = np.concatenate([np.arange(P) + P * o for o in order])
        xt = np.ascontiguousarray(x[b][perm, :].T.astype(np.float16))
        in_maps.append({
            "xt": xt, "wq": wq_h, "wk": wk_h, "wv": wv_h, "wo": wo_h,
            "maskt": masks[parity],
        })

    res = run_bass_kernel_spmd(
        nc, in_maps, core_ids=list(range(NCORES)),
        trace=_trace, **(_trace_kwargs or {}))

    out = np.empty_like(x)
    for core in range(NCORES):
        b, parity = core // 2, core % 2
        order = block_order(parity)
        core_out = res.results[core]["out"].astype(np.float32)
        for j in range(NSLOT):
            for i in range(2):
                qb = order[4 * j + i]
                out[b, P * qb:P * (qb + 1), :] = \
                    core_out[QW * j + P * i:QW * j + P * (i + 1), :]
    if _trace:
        return out, res
    return out


# revision 37
# speedup vs baseline: 1.0154x; 1.0154x over previous
"""Single-head causal attention on 8 TRN2 NeuronCores (Bass/Tile).

Problem: x [4, 2048, 1024] fp32; wq/wk/wv [1024, 128]; wo [128, 1024].
out = softmax_causal((x@wq)(x@wk)^T / sqrt(128)) @ (x@wv) @ wo

Sharding: 8 cores = 4 batches x 2 query-interleavings (zebra pattern:
core parity p takes seq blocks {4g+o0, 4g+o1} of each group g of 4
blocks). Host permutes+transposes x so each core's query slots are
contiguous; slot j = permuted q cols [512j : 512j+256] = seq blocks
{4j+order[0], 4j+order[1]}, attending permuted key prefix [0 : 512(j+1)].

Schedule notes (all perf-load-bearing):
 - x arrives seq-half-major ([128d x 1024s] granules, half 0 on the
   Scalar HWDGE queue, half 1 on Sync behind the weights) so projections
   start with the first granule; the proj phase is DMA-paced end to end.
 - Projections per seq-half (QT/KT/VT accumulate d-chunk-outer in 5 psum
   banks); psum->SBUF copies split DVE (vt, qt) / ACT (kt).
 - Attention processes slot PAIRS ({0,1}, {2,3}): shared key groups
   stream 512-wide q so matmuls are N=512 and LDWEIGHTS hides under
   streaming. Score tiles [P,1024] (2 banks, bufs=2), ONE exp per tile
   (ACT per-op overhead). Causal masks are ADDITIVE (-30000), applied by
   PE matmul-accumulation of the mask through an identity lhsT - no
   vector op in the exp->den/AV chain. den/AV skip the mask regions that
   are provably all-zero for both parities (kb1/kb3 lo-halves of diag
   groups).
 - Every multi-consumer SBUF tensor is split per half/pair (qt/kt/vt/v/
   ctxt/den/PT-per-group): Tile tracks deps at whole-tile granularity,
   so shared slabs create false WAR edges that serialize phases.
 - Output projection: 1/den scaling fused into the mandatory psum->SBUF
   copies (3 DVE / 1 ACT per block); fp16 stores on Sync; den
   partition-ified by tiny gpsimd DMAs, reciprocal after ([1,N] DVE
   reciprocal is ~8ns/elem - never do that).
"""

import numpy as np

import concourse.bass as bass
from concourse import bacc
import concourse.mybir as mybir
import concourse.tile as tile
from concourse.bass_utils import run_bass_kernel_spmd
from concourse.masks import make_identity

F32 = mybir.dt.float32
F16 = mybir.dt.float16

D_MODEL = 1024
D_HEAD = 128
SEQ = 2048
BATCH = 4
NCORES = 8
P = 128            # partitions / block size
DC = D_MODEL // P  # 8 d_model chunks
NB = SEQ // P      # 16 seq blocks
NSLOT = 4
QW = 256           # queries per slot
NQ = NSLOT * QW    # 1024 queries per core
HS = SEQ // 2      # seq half
SCALE = 1.0 / float(np.sqrt(D_HEAD))
EXP_BIAS = -3.0    # exp(scale*s - 3): keeps exp outputs < 1e4 (fp16-safe)
MASK_NEG = -30000.0


def block_order(parity: int) -> list[int]:
    order = []
    for g in range(4):
        if parity == 0:
            order += [4 * g, 4 * g + 3, 4 * g + 1, 4 * g + 2]
        else:
            order += [4 * g + 1, 4 * g + 2, 4 * g, 4 * g + 3]
    return order


def make_mask01(parity: int) -> np.ndarray:
    """0/1 keep-mask for the diagonal 512-key group of a slot,
    transposed: [512 k, 256 q]."""
    P4 = block_order(parity)[:4]
    m = np.zeros((512, 256), dtype=np.float16)
    kr = np.arange(P)[:, None]
    qc = np.arange(P)[None, :]
    tri = (kr <= qc).astype(np.float16)
    for kb2 in range(4):
        K = P4[kb2]
        for qb2 in range(2):
            Q = P4[qb2]
            blk = m[P * kb2:P * (kb2 + 1), P * qb2:P * (qb2 + 1)]
            if K < Q:
                blk[:] = 1.0
            elif K > Q:
                blk[:] = 0.0
            else:
                blk[:] = tri
    return m


def _attention_kernel(tc: tile.TileContext, xt_d, wq_d, wk_d, wv_d, wo_d,
                      maskt_d, out_d):
    nc = tc.nc

    with (
        tc.tile_pool(name="const", bufs=1) as const_pool,
        tc.tile_pool(name="big", bufs=1) as big_pool,
        tc.tile_pool(name="ptp", bufs=4) as pt_pool,
        tc.tile_pool(name="outp", bufs=3) as out_pool,
    ):
        # ---- weights + mask on the Sync HWDGE ring ----
        wq_sb = const_pool.tile([P, DC, P], F16)
        nc.sync.dma_start(out=wq_sb, in_=wq_d.rearrange("p (c h) -> p c h", h=P))
        wk_sb = const_pool.tile([P, DC, P], F16)
        nc.sync.dma_start(out=wk_sb, in_=wk_d.rearrange("p (c h) -> p c h", h=P))
        wv_sb = const_pool.tile([P, DC, P], F16)
        nc.sync.dma_start(out=wv_sb, in_=wv_d.rearrange("p (c h) -> p c h", h=P))

        # ---- x seq-half-major: half 0 on Scalar HWDGE, half 1 on Sync ----
        xt_sb = [[None, None] for _ in range(DC)]
        for h in range(2):
            eng = nc.scalar if h == 0 else nc.sync
            for c in range(DC):
                t = big_pool.tile([P, HS], F16, name=f"xt_sb{c}_{h}")
                eng.dma_start(
                    out=t, in_=xt_d[P * c:P * (c + 1), HS * h:HS * (h + 1)])
                xt_sb[c][h] = t

        # mask + wo queue behind x on Sync: not needed until attention,
        # so they must not steal early DMA bandwidth from x half 0
        maskt_sb = const_pool.tile([P, 4, QW], F16)
        nc.sync.dma_start(out=maskt_sb,
                          in_=maskt_d.rearrange("p (b q) -> p b q", q=QW))
        wo_sb = const_pool.tile([P, D_MODEL], F16)
        nc.sync.dma_start(out=wo_sb, in_=wo_d)

        # ---- constants ----
        ident = const_pool.tile([P, P], F16)
        make_identity(nc, ident)
        ones = const_pool.tile([P, 1], F16)
        nc.vector.memset(ones, 1.0)
        expbias = const_pool.tile([P, 1], F32)
        nc.vector.memset(expbias, EXP_BIAS)
        onef32 = const_pool.tile([1, 1], F32)
        nc.vector.memset(onef32, 1.0)
        # dummy exp: pulls the ACT exp table load to t=0
        actwarm = const_pool.tile([P, 1], F32)
        nc.scalar.activation(out=actwarm, in_=expbias,
                             func=mybir.ActivationFunctionType.Exp)

        # per-quarter / per-pair SBUF tensors. Split fine: Tile tracks
        # deps at whole-tile granularity, and a tile with two writer
        # engines (DVE+ACT copies) makes every reader wait on both.
        qt_h = [big_pool.tile([P, 512], F16, name=f"qt{h}") for h in range(2)]
        kt_q = [big_pool.tile([P, 512], F16, name=f"kt{q}") for q in range(4)]
        vt_q = [big_pool.tile([P, 512], F16, name=f"vt{q}") for q in range(4)]
        v_h = [big_pool.tile([P, HS], F16, name=f"v{h}") for h in range(2)]
        ctxt_p = [big_pool.tile([P, 512], F16, name=f"ctxt{a}") for a in range(2)]
        den_p = [big_pool.tile([1, 512], F32, name=f"den{a}") for a in range(2)]
        rden_p = [big_pool.tile([P, 4], F32, name=f"rden{a}") for a in range(2)]

        def kt_blk(kb):
            return kt_q[kb // 4][:, P * (kb % 4):P * (kb % 4 + 1)]

        def v_blk(kb):
            return v_h[kb // 8][:, P * (kb % 8):P * (kb % 8 + 1)]

        def proj_half(h, pj):
            """QT/KT/VT for seq half h, d-chunk outer; copies to SBUF."""
            qt_ps = pj.tile([P, 512], F32, name=f"qt_ps{h}")
            kt_ps = [pj.tile([P, 512], F32, name=f"kt_ps{h}_{i}")
                     for i in range(2)]
            vt_ps = [pj.tile([P, 512], F32, name=f"vt_ps{h}_{i}")
                     for i in range(2)]
            for c in range(DC):
                xh = xt_sb[c][h]
                xr = xh.rearrange("p (g q) -> p g q", q=QW)
                st, sp = (c == 0), (c == DC - 1)
                nc.tensor.matmul(qt_ps, lhsT=wq_sb[:, c, :],
                                 rhs=xr[:, 0:3:2, :], start=st, stop=sp,
                                 skip_group_check=True)
                for i in range(2):
                    nc.tensor.matmul(kt_ps[i], lhsT=wk_sb[:, c, :],
                                     rhs=xh[:, 512 * i:512 * (i + 1)],
                                     start=st, stop=sp, skip_group_check=True)
                for i in range(2):
                    nc.tensor.matmul(vt_ps[i], lhsT=wv_sb[:, c, :],
                                     rhs=xh[:, 512 * i:512 * (i + 1)],
                                     start=st, stop=sp, skip_group_check=True)
            # copies (vt first: transposes wait on them). For half 0,
            # balance DVE/ACT; for half 1 keep ACT free — attA's exps
            # run concurrently with these copies.
            if h == 0:
                nc.vector.tensor_copy(vt_q[0], vt_ps[0])
                nc.scalar.copy(vt_q[1], vt_ps[1])
                nc.vector.tensor_copy(kt_q[0], kt_ps[0])
                nc.scalar.copy(kt_q[1], kt_ps[1])
                nc.vector.tensor_copy(qt_h[h], qt_ps)
            else:
                nc.vector.tensor_copy(vt_q[2], vt_ps[0])
                nc.scalar.copy(vt_q[3], vt_ps[1])
                nc.vector.tensor_copy(kt_q[2], kt_ps[0])
                nc.scalar.copy(kt_q[3], kt_ps[1])
                nc.vector.tensor_copy(qt_h[h], qt_ps)

        def transposes(h, trp):
            for b in range(8):
                ptr = trp.tile([P, P], F16, tag="tr", bufs=2,
                               name=f"ptr{h}_{b}")
                nc.tensor.transpose(
                    ptr, vt_q[2 * h + b // 4][:, P * (b % 4):P * (b % 4 + 1)],
                    ident)
                nc.vector.tensor_copy(v_h[h][:, P * b:P * (b + 1)], ptr)

        def attention_pair(a, att):
            """Pair a: slots {2a, 2a+1}; pair q cols 0..511 = qt halves
            a (lo slot, cols 0..255) and hi slot (256..511)... pair q =
            [slot 2a | slot 2a+1] = qt_h[a] fully. Key groups 0..2a-1
            full, group 2a diag-masked on slot-lo, group 2a+1 slot-hi
            only."""
            jlo, jhi = 2 * a, 2 * a + 1
            nfull = jhi            # groups 0..jhi-1 stream 512q
            qt_pair = qt_h[a]
            qt_hi = qt_h[a][:, 256:512]

            den_ps = att.tile([1, 512], F32, tag="den", bufs=1,
                              name=f"den_ps{a}")
            ctx_ps = att.tile([P, 512], F32, tag="ctx", bufs=1,
                              name=f"ctx_ps{a}")
            pt = {}

            def st_exp(g):
                """Scores (+additive mask on diag regions) + exp for key
                group g -> its own PT tile. PT layouts: wide groups
                [4kb x 512q]; narrow [4kb x 256q]."""
                wide = g < nfull or g == jlo
                qw = 512 if wide else 256
                qtr = qt_pair if wide else qt_hi
                ptile = pt_pool.tile([P, 2048], F16, tag="pt",
                                     name=f"pt{a}_{g}")
                pt[g] = ptile
                diag = (g == jlo) if wide else True
                nhalf = 2 if wide else 1
                for half in range(nhalf):
                    stp = att.tile([P, 1024], F32, tag="st", bufs=2,
                                   name=f"st{a}_{g}_{half}")
                    for k2h in range(4 // nhalf):
                        k2 = half * 2 + k2h if wide else k2h
                        kb = 4 * g + k2
                        base = qw * k2h  # kb's col base in stp
                        nc.tensor.matmul(stp[:, base:base + qw],
                                         lhsT=kt_blk(kb), rhs=qtr,
                                         start=True, stop=not diag,
                                         skip_group_check=True)
                        if not diag:
                            continue
                        # additive causal mask (identity-matmul accum) on
                        # the diag slot's 256 q (= first 256 cols of the
                        # kb when wide). kb1/kb3: q0 sub-block is all
                        # -inf for both parities -> den/av skip it, so
                        # only the q1 half needs the mask rows.
                        if k2 in (0, 2):
                            nc.tensor.matmul(
                                stp[:, base:base + 256],
                                lhsT=ident, rhs=maskt_sb[:, k2, :],
                                start=False, stop=True,
                                skip_group_check=True)
                        else:
                            nc.tensor.matmul(
                                stp[:, base + P:base + 256],
                                lhsT=ident, rhs=maskt_sb[:, k2, P:QW],
                                start=False, stop=True,
                                skip_group_check=True)
                    off = 1024 * half
                    nc.scalar.activation(
                        out=ptile[:, off:off + 1024], in_=stp,
                        func=mybir.ActivationFunctionType.Exp,
                        bias=expbias, scale=SCALE)

            ngroups = jhi + 1
            ndenav = 0

            def denav(g):
                """den+AV for key group g. Skips regions that the causal
                mask provably zeroes for both parities: diag-group kb1/
                kb3 slot-lo halves."""
                nonlocal ndenav
                wide = g < nfull or g == jlo
                ptile = pt[g]
                first = (g == 0)
                last = (ndenav == ngroups - 1)
                parts = []  # (kb, pt_off, den_off, width)
                for k2 in range(4):
                    kb = 4 * g + k2
                    if wide:
                        base = 512 * k2
                        if g == jlo and k2 in (1, 3):
                            # skip slot-lo q0 sub-block (all-masked)
                            parts.append((kb, base + P, P, 512 - P))
                        else:
                            parts.append((kb, base, 0, 512))
                    else:
                        base = 256 * k2
                        if k2 in (1, 3):
                            parts.append((kb, base + P, 256 + P, P))
                        else:
                            parts.append((kb, base, 256, 256))
                for i, (kb, po, do, w) in enumerate(parts):
                    nc.tensor.matmul(den_ps[:, do:do + w], lhsT=ones,
                                     rhs=ptile[:, po:po + w],
                                     start=(first and i == 0),
                                     stop=(last and i == len(parts) - 1),
                                     skip_group_check=True)
                for i, (kb, po, do, w) in enumerate(parts):
                    nc.tensor.matmul(ctx_ps[:, do:do + w], lhsT=v_blk(kb),
                                     rhs=ptile[:, po:po + w],
                                     start=(first and i == 0),
                                     stop=(last and i == len(parts) - 1),
                                     skip_group_check=True)
                ndenav += 1

            # interleave: scores run one group ahead of den/av
            st_exp(0)
            for g in range(1, ngroups):
                st_exp(g)
                denav(g - 1)
            denav(ngroups - 1)

            # finish: ctx/den to SBUF; den partition-ified by PE
            # transpose-matmuls (lhsT = [1,128] den row, rhs = ident[0,0]
            # -> [128,1] column) into a psum tile sharing den's bank;
            # then ONE ACT Reciprocal reads psum and writes rden SBUF.
            # (The earlier gpsimd-DMA broadcast put ~2us of SWDGE latency
            # on the last pair's critical path.)
            nc.vector.tensor_copy(ctxt_p[a], ctx_ps)
            nc.vector.tensor_copy(den_p[a], den_ps)
            dent_ps = att.tile([P, 4], F32, tag="den", bufs=1,
                               name=f"dent_ps{a}")
            for qb2 in range(4):
                nc.tensor.matmul(dent_ps[:, qb2:qb2 + 1],
                                 lhsT=den_p[a][0:1, P * qb2:P * (qb2 + 1)],
                                 rhs=onef32,
                                 start=(qb2 == 0), stop=(qb2 == 3),
                                 skip_group_check=True)
            nc.vector.reciprocal(rden_p[a], dent_ps)

        def out_block(qb, att, act_heavy=False, split_store=False,
                      op_bufs=2):
            """Output projection for 128-q block qb (qt col block). 1/den
            scaling fuses into the mandatory psum->SBUF copies. While ACT
            is exp-bound (pair-A blocks) it takes 1 of 4 halves; after
            the last exp (pair-B blocks) the split is even (act_heavy).
            split_store issues each half's store as soon as it's scaled
            (for the last block, whose store is the kernel tail)."""
            a, qb2 = qb // 4, qb % 4
            rd = rden_p[a][:, qb2:qb2 + 1]
            ctxb = ctxt_p[a][:, P * qb2:P * (qb2 + 1)]
            ot = out_pool.tile([P, D_MODEL], F16, tag="ot")
            for t in range(2):
                ps = att.tile([P, 512], F32, tag="op", bufs=op_bufs,
                              name=f"op{qb}_{t}")
                nc.tensor.matmul(ps, lhsT=ctxb,
                                 rhs=wo_sb[:, 512 * t:512 * (t + 1)],
                                 start=True, stop=True,
                                 skip_group_check=True)
                on_act = (t == 1) if act_heavy else (t == 1 and qb % 2 == 0)
                if on_act:
                    nc.scalar.mul(ot[:, 512 * t:512 * (t + 1)], ps, rd)
                else:
                    nc.vector.tensor_scalar_mul(
                        ot[:, 512 * t:512 * (t + 1)], ps, rd)
                if split_store:
                    nc.sync.dma_start(
                        out=out_d[P * qb:P * (qb + 1),
                                  512 * t:512 * (t + 1)],
                        in_=ot[:, 512 * t:512 * (t + 1)])
            if not split_store:
                nc.sync.dma_start(out=out_d[P * qb:P * (qb + 1), :], in_=ot)

        # ---------------- schedule ----------------
        # PE warm-up: the HAM clock gate needs ~3.4us of sustained PE
        # activity to lift the clock from 1.2 to 2.4 GHz, and the first
        # ~4us of the kernel is DMA-wait. Burn it on dummy matmuls so the
        # projections run warm (measured ~6us of cold-clock otherwise).
        with tc.tile_pool(name="warm", bufs=1, space="PSUM") as warm:
            wps = warm.tile([P, P], F32, name="warm_ps")
            for _ in range(40):
                nc.tensor.matmul(wps, lhsT=ident, rhs=ident,
                                 start=True, stop=True,
                                 skip_group_check=True)
        # Both projections back to back (the whole proj phase is paced by
        # the single x DMA stream; pjB's matmuls fill pjA's arrival gaps).
        # PSUM budget forces the sequencing: pjA(5) -> trA(2)+pjB(5)=7 ->
        # trB(2)+attA(6)=8 -> attB(8).
        with tc.tile_pool(name="pjA", bufs=1, space="PSUM") as pjA:
            proj_half(0, pjA)
        with tc.tile_pool(name="trA", bufs=1, space="PSUM") as trA:
            transposes(0, trA)
            with tc.tile_pool(name="pjB", bufs=1, space="PSUM") as pjB:
                proj_half(1, pjB)
        with tc.tile_pool(name="trB", bufs=1, space="PSUM") as trB:
            transposes(1, trB)
            with tc.tile_pool(name="attA", bufs=1, space="PSUM") as attA:
                attention_pair(0, attA)
        with tc.tile_pool(name="attB", bufs=1, space="PSUM") as attB:
            # pair-A output blocks first: their deps (ctxt/rden) are done
            # by now, so they fill the attA->attB seam.
            for qb in range(4):
                out_block(qb, attB)
            attention_pair(1, attB)
        with tc.tile_pool(name="attB2", bufs=1, space="PSUM") as attB2:
            # pair-B outs in their own pool: the score banks are dead, so
            # op gets bufs=4 and the scale copies pipeline fully.
            for qb in range(4, 8):
                out_block(qb, attB2, act_heavy=True, split_store=(qb == 7),
                          op_bufs=4)


_NC_CACHE = None


def build_nc() -> bass.Bass:
    global _NC_CACHE
    if _NC_CACHE is not None:
        return _NC_CACHE
    nc = bacc.Bacc("TRN2", target_bir_lowering=False, debug=False)
    xt_d = nc.dram_tensor("xt", [D_MODEL, SEQ], F16, kind="ExternalInput").ap()
    wq_d = nc.dram_tensor("wq", [P, DC * D_HEAD], F16, kind="ExternalInput").ap()
    wk_d = nc.dram_tensor("wk", [P, DC * D_HEAD], F16, kind="ExternalInput").ap()
    wv_d = nc.dram_tensor("wv", [P, DC * D_HEAD], F16, kind="ExternalInput").ap()
    wo_d = nc.dram_tensor("wo", [D_HEAD, D_MODEL], F16, kind="ExternalInput").ap()
    maskt_d = nc.dram_tensor("maskt", [P, 4 * QW], F16, kind="ExternalInput").ap()
    out_d = nc.dram_tensor("out", [NQ, D_MODEL], F16, kind="ExternalOutput").ap()
    with tile.TileContext(nc) as tc:
        _attention_kernel(tc, xt_d, wq_d, wk_d, wv_d, wo_d, maskt_d, out_d)
    nc.compile()
    _NC_CACHE = nc
    return nc


def _chunk_major(w):
    """[1024, 128] -> [128, 8*128]: row p holds chunks c of w[128c+p, :]."""
    return np.ascontiguousarray(
        w.reshape(DC, P, D_HEAD).transpose(1, 0, 2).reshape(P, DC * D_HEAD))


def kernel(x, wq, wk, wv, wo, _trace=False, _trace_kwargs=None):
    x = np.asarray(x, dtype=np.float32)
    wq_h = _chunk_major(np.asarray(wq, dtype=np.float32).astype(np.float16))
    wk_h = _chunk_major(np.asarray(wk, dtype=np.float32).astype(np.float16))
    wv_h = _chunk_major(np.asarray(wv, dtype=np.float32).astype(np.float16))
    wo_h = np.ascontiguousarray(np.asarray(wo, dtype=np.float32).astype(np.float16))

    nc = build_nc()

    masks = {}
    for p in (0, 1):
        m = (1.0 - make_mask01(p).astype(np.float32)) * MASK_NEG  # additive
        m = m.astype(np.float16)  # [512 k, 256 q]
        masks[p] = np.ascontiguousarray(
            m.reshape(4, P, QW).transpose(1, 0, 2).reshape(P, 4 * QW))
    in_maps = []
    for core in range(NCORES):
        b, parity = core // 2, core % 2
        order = block_order(parity)
        perm = np.concatenate([np.arange(P) + P * o for o in order])
        xt = np.ascontiguousarray(x[b][perm, :].T.astype(np.float16))
        in_maps.append({
            "xt": xt, "wq": wq_h, "wk": wk_h, "wv": wv_h, "wo": wo_h,
            "maskt": masks[parity],
        })

    res = run_bass_kernel_spmd(
        nc, in_maps, core_ids=list(range(NCORES)),
        trace=_trace, **(_trace_kwargs or {}))

    out = np.empty_like(x)
    for core in range(NCORES):
        b, parity = core // 2, core % 2
        order = block_order(parity)
        core_out = res.results[core]["out"].astype(np.float32)
        for j in range(NSLOT):
            for i in range(2):
                qb = order[4 * j + i]
                out[b, P * qb:P * (qb + 1), :] = \
                    core_out[QW * j + P * i:QW * j + P * (i + 1), :]
    if _trace:
        return out, res
    return out


# revision 39
# speedup vs baseline: 1.0321x; 1.0164x over previous
"""Single-head causal attention on 8 TRN2 NeuronCores (Bass/Tile).

Problem: x [4, 2048, 1024] fp32; wq/wk/wv [1024, 128]; wo [128, 1024].
out = softmax_causal((x@wq)(x@wk)^T / sqrt(128)) @ (x@wv) @ wo

Sharding: 8 cores = 4 batches x 2 query-interleavings (zebra pattern:
core parity p takes seq blocks {4g+o0, 4g+o1} of each group g of 4
blocks). Host permutes+transposes x so each core's query slots are
contiguous; slot j = permuted q cols [512j : 512j+256] = seq blocks
{4j+order[0], 4j+order[1]}, attending permuted key prefix [0 : 512(j+1)].

Schedule notes (all perf-load-bearing):
 - x arrives seq-half-major ([128d x 1024s] granules, half 0 on the
   Scalar HWDGE queue, half 1 on Sync behind the weights) so projections
   start with the first granule; the proj phase is DMA-paced end to end.
 - Projections per seq-half (QT/KT/VT accumulate d-chunk-outer in 5 psum
   banks); psum->SBUF copies split DVE (vt, qt) / ACT (kt).
 - Attention processes slot PAIRS ({0,1}, {2,3}): shared key groups
   stream 512-wide q so matmuls are N=512 and LDWEIGHTS hides under
   streaming. Score tiles [P,1024] (2 banks, bufs=2), ONE exp per tile
   (ACT per-op overhead). Causal masks are ADDITIVE (-30000), applied by
   PE matmul-accumulation of the mask through an identity lhsT - no
   vector op in the exp->den/AV chain. den/AV skip the mask regions that
   are provably all-zero for both parities (kb1/kb3 lo-halves of diag
   groups).
 - Every multi-consumer SBUF tensor is split per half/pair (qt/kt/vt/v/
   ctxt/den/PT-per-group): Tile tracks deps at whole-tile granularity,
   so shared slabs create false WAR edges that serialize phases.
 - Output projection: 1/den scaling fused into the mandatory psum->SBUF
   copies (3 DVE / 1 ACT per block); fp16 stores on Sync; den
   partition-ified by tiny gpsimd DMAs, reciprocal after ([1,N] DVE
   reciprocal is ~8ns/elem - never do that).
"""

import numpy as np

import concourse.bass as bass
from concourse import bacc
import concourse.mybir as mybir
import concourse.tile as tile
from concourse.bass_utils import run_bass_kernel_spmd
from concourse.masks import make_identity

F32 = mybir.dt.float32
F16 = mybir.dt.float16

D_MODEL = 1024
D_HEAD = 128
SEQ = 2048
BATCH = 4
NCORES = 8
P = 128            # partitions / block size
DC = D_MODEL // P  # 8 d_model chunks
NB = SEQ // P      # 16 seq blocks
NSLOT = 4
QW = 256           # queries per slot
NQ = NSLOT * QW    # 1024 queries per core
HS = SEQ // 2      # seq half
SCALE = 1.0 / float(np.sqrt(D_HEAD))
EXP_BIAS = -3.0    # exp(scale*s - 3): keeps exp outputs < 1e4 (fp16-safe)
MASK_NEG = -30000.0


def block_order(parity: int) -> list[int]:
    order = []
    for g in range(4):
        if parity == 0:
            order += [4 * g, 4 * g + 3, 4 * g + 1, 4 * g + 2]
        else:
            order += [4 * g + 1, 4 * g + 2, 4 * g, 4 * g + 3]
    return order


def make_mask01(parity: int) -> np.ndarray:
    """0/1 keep-mask for the diagonal 512-key group of a slot,
    transposed: [512 k, 256 q]."""
    P4 = block_order(parity)[:4]
    m = np.zeros((512, 256), dtype=np.float16)
    kr = np.arange(P)[:, None]
    qc = np.arange(P)[None, :]
    tri = (kr <= qc).astype(np.float16)
    for kb2 in range(4):
        K = P4[kb2]
        for qb2 in range(2):
            Q = P4[qb2]
            blk = m[P * kb2:P * (kb2 + 1), P * qb2:P * (qb2 + 1)]
            if K < Q:
                blk[:] = 1.0
            elif K > Q:
                blk[:] = 0.0
            else:
                blk[:] = tri
    return m


def _attention_kernel(tc: tile.TileContext, xt_d, wq_d, wk_d, wv_d, wo_d,
                      maskt_d, out_d):
    nc = tc.nc

    with (
        tc.tile_pool(name="const", bufs=1) as const_pool,
        tc.tile_pool(name="big", bufs=1) as big_pool,
        tc.tile_pool(name="ptp", bufs=4) as pt_pool,
        tc.tile_pool(name="outp", bufs=3) as out_pool,
    ):
        # ---- weights + mask on the Sync HWDGE ring ----
        wq_sb = const_pool.tile([P, DC, P], F16)
        nc.sync.dma_start(out=wq_sb, in_=wq_d.rearrange("p (c h) -> p c h", h=P))
        wk_sb = const_pool.tile([P, DC, P], F16)
        nc.sync.dma_start(out=wk_sb, in_=wk_d.rearrange("p (c h) -> p c h", h=P))
        wv_sb = const_pool.tile([P, DC, P], F16)
        nc.sync.dma_start(out=wv_sb, in_=wv_d.rearrange("p (c h) -> p c h", h=P))

        # ---- x seq-half-major: half 0 on Scalar HWDGE, half 1 on Sync ----
        xt_sb = [[None, None] for _ in range(DC)]
        for h in range(2):
            eng = nc.scalar if h == 0 else nc.sync
            for c in range(DC):
                t = big_pool.tile([P, HS], F16, name=f"xt_sb{c}_{h}")
                eng.dma_start(
                    out=t, in_=xt_d[P * c:P * (c + 1), HS * h:HS * (h + 1)])
                xt_sb[c][h] = t

        # mask + wo queue behind x on Sync: not needed until attention,
        # so they must not steal early DMA bandwidth from x half 0
        maskt_sb = const_pool.tile([P, 4, QW], F16)
        nc.sync.dma_start(out=maskt_sb,
                          in_=maskt_d.rearrange("p (b q) -> p b q", q=QW))
        wo_sb = const_pool.tile([P, D_MODEL], F16)
        nc.sync.dma_start(out=wo_sb, in_=wo_d)

        # ---- constants ----
        ident = const_pool.tile([P, P], F16)
        make_identity(nc, ident)
        ones = const_pool.tile([P, 1], F16)
        nc.vector.memset(ones, 1.0)
        expbias = const_pool.tile([P, 1], F32)
        nc.vector.memset(expbias, EXP_BIAS)
        onef32 = const_pool.tile([1, 1], F32)
        nc.vector.memset(onef32, 1.0)
        # dummy exp: pulls the ACT exp table load to t=0
        actwarm = const_pool.tile([P, 1], F32)
        nc.scalar.activation(out=actwarm, in_=expbias,
                             func=mybir.ActivationFunctionType.Exp)

        # per-quarter / per-pair SBUF tensors. Split fine: Tile tracks
        # deps at whole-tile granularity, and a tile with two writer
        # engines (DVE+ACT copies) makes every reader wait on both.
        qt_h = [big_pool.tile([P, 512], F16, name=f"qt{h}") for h in range(2)]
        kt_q = [big_pool.tile([P, 512], F16, name=f"kt{q}") for q in range(4)]
        vt_q = [big_pool.tile([P, 512], F16, name=f"vt{q}") for q in range(4)]
        v_h = [big_pool.tile([P, HS], F16, name=f"v{h}") for h in range(2)]
        ctxt_p = [big_pool.tile([P, 512], F16, name=f"ctxt{a}") for a in range(2)]
        den_p = [big_pool.tile([1, 512], F32, name=f"den{a}") for a in range(2)]
        rden_p = [big_pool.tile([P, 4], F32, name=f"rden{a}") for a in range(2)]

        def kt_blk(kb):
            return kt_q[kb // 4][:, P * (kb % 4):P * (kb % 4 + 1)]

        def v_blk(kb):
            return v_h[kb // 8][:, P * (kb % 8):P * (kb % 8 + 1)]

        def proj_half(h, pj):
            """QT/KT/VT for seq half h, d-chunk outer; copies to SBUF."""
            qt_ps = pj.tile([P, 512], F32, name=f"qt_ps{h}")
            kt_ps = [pj.tile([P, 512], F32, name=f"kt_ps{h}_{i}")
                     for i in range(2)]
            vt_ps = [pj.tile([P, 512], F32, name=f"vt_ps{h}_{i}")
                     for i in range(2)]
            for c in range(DC):
                xh = xt_sb[c][h]
                xr = xh.rearrange("p (g q) -> p g q", q=QW)
                st, sp = (c == 0), (c == DC - 1)
                nc.tensor.matmul(qt_ps, lhsT=wq_sb[:, c, :],
                                 rhs=xr[:, 0:3:2, :], start=st, stop=sp,
                                 skip_group_check=True)
                for i in range(2):
                    nc.tensor.matmul(kt_ps[i], lhsT=wk_sb[:, c, :],
                                     rhs=xh[:, 512 * i:512 * (i + 1)],
                                     start=st, stop=sp, skip_group_check=True)
                for i in range(2):
                    nc.tensor.matmul(vt_ps[i], lhsT=wv_sb[:, c, :],
                                     rhs=xh[:, 512 * i:512 * (i + 1)],
                                     start=st, stop=sp, skip_group_check=True)
            # copies (vt first: transposes wait on them). For half 0,
            # balance DVE/ACT; for half 1 keep ACT free — attA's exps
            # run concurrently with these copies.
            if h == 0:
                nc.vector.tensor_copy(vt_q[0], vt_ps[0])
                nc.scalar.copy(vt_q[1], vt_ps[1])
                nc.vector.tensor_copy(kt_q[0], kt_ps[0])
                nc.scalar.copy(kt_q[1], kt_ps[1])
                nc.vector.tensor_copy(qt_h[h], qt_ps)
            else:
                nc.vector.tensor_copy(vt_q[2], vt_ps[0])
                nc.scalar.copy(vt_q[3], vt_ps[1])
                nc.vector.tensor_copy(kt_q[2], kt_ps[0])
                nc.scalar.copy(kt_q[3], kt_ps[1])
                nc.vector.tensor_copy(qt_h[h], qt_ps)

        def transposes(h, trp):
            for b in range(8):
                ptr = trp.tile([P, P], F16, tag="tr", bufs=2,
                               name=f"ptr{h}_{b}")
                nc.tensor.transpose(
                    ptr, vt_q[2 * h + b // 4][:, P * (b % 4):P * (b % 4 + 1)],
                    ident)
                nc.vector.tensor_copy(v_h[h][:, P * b:P * (b + 1)], ptr)

        def attention_pair(a, att):
            """Pair a: slots {2a, 2a+1}; pair q cols 0..511 = qt halves
            a (lo slot, cols 0..255) and hi slot (256..511)... pair q =
            [slot 2a | slot 2a+1] = qt_h[a] fully. Key groups 0..2a-1
            full, group 2a diag-masked on slot-lo, group 2a+1 slot-hi
            only."""
            jlo, jhi = 2 * a, 2 * a + 1
            nfull = jhi            # groups 0..jhi-1 stream 512q
            qt_pair = qt_h[a]
            qt_hi = qt_h[a][:, 256:512]

            den_ps = att.tile([1, 512], F32, tag="den", bufs=1,
                              name=f"den_ps{a}")
            ctx_ps = att.tile([P, 512], F32, tag="ctx", bufs=1,
                              name=f"ctx_ps{a}")
            pt = {}

            def st_exp(g):
                """Scores (+additive mask on diag regions) + exp for key
                group g -> its own PT tile. PT layouts: wide groups
                [4kb x 512q]; narrow [4kb x 256q]."""
                wide = g < nfull or g == jlo
                qw = 512 if wide else 256
                qtr = qt_pair if wide else qt_hi
                ptile = pt_pool.tile([P, 2048], F16, tag="pt",
                                     name=f"pt{a}_{g}")
                pt[g] = ptile
                diag = (g == jlo) if wide else True
                nhalf = 2 if wide else 1
                for half in range(nhalf):
                    stp = att.tile([P, 1024], F32, tag="st", bufs=2,
                                   name=f"st{a}_{g}_{half}")
                    for k2h in range(4 // nhalf):
                        k2 = half * 2 + k2h if wide else k2h
                        kb = 4 * g + k2
                        base = qw * k2h  # kb's col base in stp
                        nc.tensor.matmul(stp[:, base:base + qw],
                                         lhsT=kt_blk(kb), rhs=qtr,
                                         start=True, stop=not diag,
                                         skip_group_check=True)
                        if not diag:
                            continue
                        # additive causal mask (identity-matmul accum) on
                        # the diag slot's 256 q (= first 256 cols of the
                        # kb when wide). kb1/kb3: q0 sub-block is all
                        # -inf for both parities -> den/av skip it, so
                        # only the q1 half needs the mask rows.
                        if k2 in (0, 2):
                            nc.tensor.matmul(
                                stp[:, base:base + 256],
                                lhsT=ident, rhs=maskt_sb[:, k2, :],
                                start=False, stop=True,
                                skip_group_check=True)
                        else:
                            nc.tensor.matmul(
                                stp[:, base + P:base + 256],
                                lhsT=ident, rhs=maskt_sb[:, k2, P:QW],
                                start=False, stop=True,
                                skip_group_check=True)
                    off = 1024 * half
                    nc.scalar.activation(
                        out=ptile[:, off:off + 1024], in_=stp,
                        func=mybir.ActivationFunctionType.Exp,
                        bias=expbias, scale=SCALE)

            ngroups = jhi + 1
            ndenav = 0

            def denav(g):
                """den+AV for key group g. Skips regions that the causal
                mask provably zeroes for both parities: diag-group kb1/
                kb3 slot-lo halves."""
                nonlocal ndenav
                wide = g < nfull or g == jlo
                ptile = pt[g]
                first = (g == 0)
                last = (ndenav == ngroups - 1)
                parts = []  # (kb, pt_off, den_off, width)
                for k2 in range(4):
                    kb = 4 * g + k2
                    if wide:
                        base = 512 * k2
                        if g == jlo and k2 in (1, 3):
                            # skip slot-lo q0 sub-block (all-masked)
                            parts.append((kb, base + P, P, 512 - P))
                        else:
                            parts.append((kb, base, 0, 512))
                    else:
                        base = 256 * k2
                        if k2 in (1, 3):
                            parts.append((kb, base + P, 256 + P, P))
                        else:
                            parts.append((kb, base, 256, 256))
                for i, (kb, po, do, w) in enumerate(parts):
                    nc.tensor.matmul(den_ps[:, do:do + w], lhsT=ones,
                                     rhs=ptile[:, po:po + w],
                                     start=(first and i == 0),
                                     stop=(last and i == len(parts) - 1),
                                     skip_group_check=True)
                for i, (kb, po, do, w) in enumerate(parts):
                    nc.tensor.matmul(ctx_ps[:, do:do + w], lhsT=v_blk(kb),
                                     rhs=ptile[:, po:po + w],
                                     start=(first and i == 0),
                                     stop=(last and i == len(parts) - 1),
                                     skip_group_check=True)
                ndenav += 1

            # interleave: scores run one group ahead of den/av
            st_exp(0)
            for g in range(1, ngroups):
                st_exp(g)
                denav(g - 1)
            denav(ngroups - 1)

            # finish: ctx/den to SBUF; den partition-ified by PE
            # transpose-matmuls (lhsT = [1,128] den row, rhs = ident[0,0]
            # -> [128,1] column) into a psum tile sharing den's bank;
            # then ONE ACT Reciprocal reads psum and writes rden SBUF.
            # (The earlier gpsimd-DMA broadcast put ~2us of SWDGE latency
            # on the last pair's critical path.)
            nc.vector.tensor_copy(ctxt_p[a], ctx_ps)
            nc.vector.tensor_copy(den_p[a], den_ps)
            dent_ps = att.tile([P, 4], F32, tag="den", bufs=1,
                               name=f"dent_ps{a}")
            for qb2 in range(4):
                nc.tensor.matmul(dent_ps[:, qb2:qb2 + 1],
                                 lhsT=den_p[a][0:1, P * qb2:P * (qb2 + 1)],
                                 rhs=onef32,
                                 start=(qb2 == 0), stop=(qb2 == 3),
                                 skip_group_check=True)
            nc.vector.reciprocal(rden_p[a], dent_ps)

        def out_block(qb, att, act_heavy=False, split_store=False,
                      op_bufs=2):
            """Output projection for 128-q block qb (qt col block). 1/den
            scaling fuses into the mandatory psum->SBUF copies. While ACT
            is exp-bound (pair-A blocks) it takes 1 of 4 halves; after
            the last exp (pair-B blocks) the split is even (act_heavy).
            split_store issues each half's store as soon as it's scaled
            (for the last block, whose store is the kernel tail)."""
            a, qb2 = qb // 4, qb % 4
            rd = rden_p[a][:, qb2:qb2 + 1]
            ctxb = ctxt_p[a][:, P * qb2:P * (qb2 + 1)]
            ot = out_pool.tile([P, D_MODEL], F16, tag="ot")
            for t in range(2):
                ps = att.tile([P, 512], F32, tag="op", bufs=op_bufs,
                              name=f"op{qb}_{t}")
                nc.tensor.matmul(ps, lhsT=ctxb,
                                 rhs=wo_sb[:, 512 * t:512 * (t + 1)],
                                 start=True, stop=True,
                                 skip_group_check=True)
                on_act = (t == 1) if act_heavy else (t == 1 and qb % 2 == 0)
                if on_act:
                    nc.scalar.mul(ot[:, 512 * t:512 * (t + 1)], ps, rd)
                else:
                    nc.vector.tensor_scalar_mul(
                        ot[:, 512 * t:512 * (t + 1)], ps, rd)
                if split_store:
                    nc.sync.dma_start(
                        out=out_d[P * qb:P * (qb + 1),
                                  512 * t:512 * (t + 1)],
                        in_=ot[:, 512 * t:512 * (t + 1)])
            if not split_store:
                nc.sync.dma_start(out=out_d[P * qb:P * (qb + 1), :], in_=ot)

        # ---------------- schedule ----------------
        # PE warm-up: the HAM clock gate needs ~3.4us of sustained PE
        # activity to lift the clock from 1.2 to 2.4 GHz, and the first
        # ~4us of the kernel is DMA-wait. Burn it on dummy matmuls so the
        # projections run warm (measured ~6us of cold-clock otherwise).
        with tc.tile_pool(name="warm", bufs=1, space="PSUM") as warm:
            wps = warm.tile([P, P], F32, name="warm_ps")
            for _ in range(40):
                nc.tensor.matmul(wps, lhsT=ident, rhs=ident,
                                 start=True, stop=True,
                                 skip_group_check=True)
        # Both projections back to back (the whole proj phase is paced by
        # the single x DMA stream; pjB's matmuls fill pjA's arrival gaps).
        # PSUM budget forces the sequencing: pjA(5) -> trA(2)+pjB(5)=7 ->
        # trB(2)+attA(6)=8 -> attB(8).
        with tc.tile_pool(name="pjA", bufs=1, space="PSUM") as pjA:
            proj_half(0, pjA)
        with tc.tile_pool(name="trA", bufs=1, space="PSUM") as trA:
            transposes(0, trA)
            with tc.tile_pool(name="pjB", bufs=1, space="PSUM") as pjB:
                proj_half(1, pjB)
        with tc.tile_pool(name="trB", bufs=1, space="PSUM") as trB:
            transposes(1, trB)
            with tc.tile_pool(name="attA", bufs=1, space="PSUM") as attA:
                attention_pair(0, attA)
        with tc.tile_pool(name="attB", bufs=1, space="PSUM") as attB:
            # pair-A output blocks first: their deps (ctxt/rden) are done
            # by now, so they fill the attA->attB seam. (Moving them
            # after pair B corrupts results — don't.)
            for qb in range(4):
                out_block(qb, attB)
            attention_pair(1, attB)
        with tc.tile_pool(name="attB2", bufs=1, space="PSUM") as attB2:
            # pair-B outs in their own pool: the score banks are dead, so
            # op gets bufs=4 and the scale copies pipeline fully.
            for qb in range(4, 8):
                out_block(qb, attB2, act_heavy=True, split_store=(qb == 7),
                          op_bufs=4)


_NC_CACHE = None


def build_nc() -> bass.Bass:
    global _NC_CACHE
    if _NC_CACHE is not None:
        return _NC_CACHE
    nc = bacc.Bacc("TRN2", target_bir_lowering=False, debug=False)
    xt_d = nc.dram_tensor("xt", [D_MODEL, SEQ], F16, kind="ExternalInput").ap()
    wq_d = nc.dram_tensor("wq", [P, DC * D_HEAD], F16, kind="ExternalInput").ap()
    wk_d = nc.dram_tensor("wk", [P, DC * D_HEAD], F16, kind="ExternalInput").ap()
    wv_d = nc.dram_tensor("wv", [P, DC * D_HEAD], F16, kind="ExternalInput").ap()
    wo_d = nc.dram_tensor("wo", [D_HEAD, D_MODEL], F16, kind="ExternalInput").ap()
    maskt_d = nc.dram_tensor("maskt", [P, 4 * QW], F16, kind="ExternalInput").ap()
    out_d = nc.dram_tensor("out", [NQ, D_MODEL], F16, kind="ExternalOutput").ap()
    with tile.TileContext(nc) as tc:
        _attention_kernel(tc, xt_d, wq_d, wk_d, wv_d, wo_d, maskt_d, out_d)
    nc.compile()
    _NC_CACHE = nc
    return nc


def _chunk_major(w):
    """[1024, 128] -> [128, 8*128]: row p holds chunks c of w[128c+p, :]."""
    return np.ascontiguousarray(
        w.reshape(DC, P, D_HEAD).transpose(1, 0, 2).reshape(P, DC * D_HEAD))


def kernel(x, wq, wk, wv, wo, _trace=False, _trace_kwargs=None):
    x = np.asarray(x, dtype=np.float32)
    wq_h = _chunk_major(np.asarray(wq, dtype=np.float32).astype(np.float16))
    wk_h = _chunk_major(np.asarray(wk, dtype=np.float32).astype(np.float16))
    wv_h = _chunk_major(np.asarray(wv, dtype=np.float32).astype(np.float16))
    wo_h = np.ascontiguousarray(np.asarray(wo, dtype=np.float32).astype(np.float16))

    nc = build_nc()

    masks = {}
    for p in (0, 1):
        m = (1.0 - make_mask01(p).astype(np.float32)) * MASK_NEG  # additive
        m = m.astype(np.float16)  # [512 k, 256 q]
        masks[p] = np.ascontiguousarray(
            m.reshape(4, P, QW).transpose(1, 0, 2).reshape(P, 4 * QW))
    in_maps = []
    for core in range(NCORES):
        b, parity = core // 2, core % 2
        order = block_order(parity)
        perm = np.concatenate([np.arange(P) + P * o for o in order])
        xt = np.ascontiguousarray(x[b][perm, :].T.astype(np.float16))
        in_maps.append({
            "xt": xt, "wq": wq_h, "wk": wk_h, "wv": wv_h, "wo": wo_h,
            "maskt": masks[parity],
        })

    res = run_bass_kernel_spmd(
        nc, in_maps, core_ids=list(range(NCORES)),
        trace=_trace, **(_trace_kwargs or {}))

    out = np.empty_like(x)
    for core in range(NCORES):
        b, parity = core // 2, core % 2
        order = block_order(parity)
        core_out = res.results[core]["out"].astype(np.float32)
        for j in range(NSLOT):
            for i in range(2):
                qb = order[4 * j + i]
                out[b, P * qb:P * (qb + 1), :] = \
                    core_out[QW * j + P * i:QW * j + P * (i + 1), :]
    if _trace:
        return out, res
    return out


# revision 40
# speedup vs baseline: 1.0410x; 1.0086x over previous
"""Single-head causal attention on 8 TRN2 NeuronCores (Bass/Tile).

Problem: x [4, 2048, 1024] fp32; wq/wk/wv [1024, 128]; wo [128, 1024].
out = softmax_causal((x@wq)(x@wk)^T / sqrt(128)) @ (x@wv) @ wo

Sharding: 8 cores = 4 batches x 2 query-interleavings (zebra pattern:
core parity p takes seq blocks {4g+o0, 4g+o1} of each group g of 4
blocks). Host permutes+transposes x so each core's query slots are
contiguous; slot j = permuted q cols [512j : 512j+256] = seq blocks
{4j+order[0], 4j+order[1]}, attending permuted key prefix [0 : 512(j+1)].

Schedule notes (all perf-load-bearing; measured on HW, 79.5us -> 66.8us):
 - 40 dummy PE matmuls at t=0: the HAM clock gate needs ~3.4us of
   sustained PE activity to lift the clock 1.2 -> 2.4 GHz, and the first
   ~4us is DMA-wait anyway (saved ~6us of cold-clock projections).
 - x arrives seq-half-major ([128d x 1024s] granules, half 0 on the
   Scalar HWDGE queue, half 1 on Sync behind the weights; mask/wo queue
   last) so projections start with the first granule; pjA+pjB run back
   to back, DMA-paced end to end, with V-transposes nested between.
 - Projections per seq-half (QT/KT/VT accumulate d-chunk-outer in 5 psum
   banks); psum->SBUF copies split across DVE and ACT, into per-quarter
   kt/vt tiles so no reader ever waits the other engine's copy.
 - Attention processes slot PAIRS ({0,1}, {2,3}): shared key groups
   stream 512-wide q so matmuls are N=512 and LDWEIGHTS hides under
   streaming. Score tiles [P,1024] (2 banks, bufs=2), ONE exp per tile
   (ACT is ~1.07ns/col + ~180ns/op; per-kb exps throttle the pipeline,
   a 4-bank slab with bufs=1 stalls it). Causal masks are ADDITIVE
   (-30000), applied by PE matmul-accumulation through an identity lhsT
   - no vector op in the exp->den/AV chain. den/AV skip the regions
   that are all-masked for both parities (kb1/kb3 q0-blocks of diag
   groups).
 - Every multi-consumer SBUF tensor is split per half/pair/group
   (qt/kt/vt/v/ctxt/den/PT-per-group): Tile tracks deps at whole-tile
   granularity, so shared slabs create false WAR edges that serialize
   phases (this was worth ~9us).
 - den is partition-ified by four [1,128]->[128,1] PE transpose-matmuls
   (lhsT = den row, rhs = [1,1] one) into a psum tile sharing den's
   bank, then one [128,4] DVE reciprocal. (gpsimd-DMA broadcast costs
   ~2us of SWDGE latency on the last pair's critical path; [1,N] DVE
   reciprocal is ~8ns/elem single-lane - never do either.)
 - Output projection: 1/den scaling fused into the mandatory psum->SBUF
   copies; pair-A blocks 3 DVE / 1 ACT (ACT is exp-bound), pair-B
   blocks split evenly in their own psum pool with op bufs=4; fp16
   stores on Sync (host upconverts); the last block stores per-half.
 - Emission order pair-A outs BEFORE pair-B attention is load-bearing
   for correctness as well as the seam fill: moving them after pair B
   produced corrupt results (9e-2 rel err) - do not reorder.
"""

import numpy as np

import concourse.bass as bass
from concourse import bacc
import concourse.mybir as mybir
import concourse.tile as tile
from concourse.bass_utils import run_bass_kernel_spmd
from concourse.masks import make_identity

F32 = mybir.dt.float32
F16 = mybir.dt.float16

D_MODEL = 1024
D_HEAD = 128
SEQ = 2048
BATCH = 4
NCORES = 8
P = 128            # partitions / block size
DC = D_MODEL // P  # 8 d_model chunks
NB = SEQ // P      # 16 seq blocks
NSLOT = 4
QW = 256           # queries per slot
NQ = NSLOT * QW    # 1024 queries per core
HS = SEQ // 2      # seq half
SCALE = 1.0 / float(np.sqrt(D_HEAD))
EXP_BIAS = -3.0    # exp(scale*s - 3): keeps exp outputs < 1e4 (fp16-safe)
MASK_NEG = -30000.0


def block_order(parity: int) -> list[int]:
    order = []
    for g in range(4):
        if parity == 0:
            order += [4 * g, 4 * g + 3, 4 * g + 1, 4 * g + 2]
        else:
            order += [4 * g + 1, 4 * g + 2, 4 * g, 4 * g + 3]
    return order


def make_mask01(parity: int) -> np.ndarray:
    """0/1 keep-mask for the diagonal 512-key group of a slot,
    transposed: [512 k, 256 q]."""
    P4 = block_order(parity)[:4]
    m = np.zeros((512, 256), dtype=np.float16)
    kr = np.arange(P)[:, None]
    qc = np.arange(P)[None, :]
    tri = (kr <= qc).astype(np.float16)
    for kb2 in range(4):
        K = P4[kb2]
        for qb2 in range(2):
            Q = P4[qb2]
            blk = m[P * kb2:P * (kb2 + 1), P * qb2:P * (qb2 + 1)]
            if K < Q:
                blk[:] = 1.0
            elif K > Q:
                blk[:] = 0.0
            else:
                blk[:] = tri
    return m


def _attention_kernel(tc: tile.TileContext, xt_d, wq_d, wk_d, wv_d, wo_d,
                      maskt_d, out_d):
    nc = tc.nc

    with (
        tc.tile_pool(name="const", bufs=1) as const_pool,
        tc.tile_pool(name="big", bufs=1) as big_pool,
        tc.tile_pool(name="ptp", bufs=4) as pt_pool,
        tc.tile_pool(name="outp", bufs=3) as out_pool,
    ):
        # ---- weights + mask on the Sync HWDGE ring ----
        wq_sb = const_pool.tile([P, DC, P], F16)
        nc.sync.dma_start(out=wq_sb, in_=wq_d.rearrange("p (c h) -> p c h", h=P))
        wk_sb = const_pool.tile([P, DC, P], F16)
        nc.sync.dma_start(out=wk_sb, in_=wk_d.rearrange("p (c h) -> p c h", h=P))
        wv_sb = const_pool.tile([P, DC, P], F16)
        nc.sync.dma_start(out=wv_sb, in_=wv_d.rearrange("p (c h) -> p c h", h=P))

        # ---- x seq-half-major: half 0 on Scalar HWDGE, half 1 on Sync ----
        xt_sb = [[None, None] for _ in range(DC)]
        for h in range(2):
            eng = nc.scalar if h == 0 else nc.sync
            for c in range(DC):
                t = big_pool.tile([P, HS], F16, name=f"xt_sb{c}_{h}")
                eng.dma_start(
                    out=t, in_=xt_d[P * c:P * (c + 1), HS * h:HS * (h + 1)])
                xt_sb[c][h] = t

        # mask + wo queue behind x on Sync: not needed until attention,
        # so they must not steal early DMA bandwidth from x half 0
        maskt_sb = const_pool.tile([P, 4, QW], F16)
        nc.sync.dma_start(out=maskt_sb,
                          in_=maskt_d.rearrange("p (b q) -> p b q", q=QW))
        wo_sb = const_pool.tile([P, D_MODEL], F16)
        nc.sync.dma_start(out=wo_sb, in_=wo_d)

        # ---- constants ----
        ident = const_pool.tile([P, P], F16)
        make_identity(nc, ident)
        ones = const_pool.tile([P, 1], F16)
        nc.vector.memset(ones, 1.0)
        expbias = const_pool.tile([P, 1], F32)
        nc.vector.memset(expbias, EXP_BIAS)
        onef32 = const_pool.tile([1, 1], F32)
        nc.vector.memset(onef32, 1.0)
        # dummy exp: pulls the ACT exp table load to t=0
        actwarm = const_pool.tile([P, 1], F32)
        nc.scalar.activation(out=actwarm, in_=expbias,
                             func=mybir.ActivationFunctionType.Exp)

        # per-quarter / per-pair SBUF tensors. Split fine: Tile tracks
        # deps at whole-tile granularity, and a tile with two writer
        # engines (DVE+ACT copies) makes every reader wait on both.
        qt_h = [big_pool.tile([P, 512], F16, name=f"qt{h}") for h in range(2)]
        kt_q = [big_pool.tile([P, 512], F16, name=f"kt{q}") for q in range(4)]
        vt_q = [big_pool.tile([P, 512], F16, name=f"vt{q}") for q in range(4)]
        v_h = [big_pool.tile([P, HS], F16, name=f"v{h}") for h in range(2)]
        ctxt_p = [big_pool.tile([P, 512], F16, name=f"ctxt{a}") for a in range(2)]
        den_p = [big_pool.tile([1, 512], F32, name=f"den{a}") for a in range(2)]
        rden_p = [big_pool.tile([P, 4], F32, name=f"rden{a}") for a in range(2)]

        def kt_blk(kb):
            return kt_q[kb // 4][:, P * (kb % 4):P * (kb % 4 + 1)]

        def v_blk(kb):
            return v_h[kb // 8][:, P * (kb % 8):P * (kb % 8 + 1)]

        def proj_half(h, pj):
            """QT/KT/VT for seq half h, d-chunk outer; copies to SBUF."""
            qt_ps = pj.tile([P, 512], F32, name=f"qt_ps{h}")
            kt_ps = [pj.tile([P, 512], F32, name=f"kt_ps{h}_{i}")
                     for i in range(2)]
            vt_ps = [pj.tile([P, 512], F32, name=f"vt_ps{h}_{i}")
                     for i in range(2)]
            for c in range(DC):
                xh = xt_sb[c][h]
                xr = xh.rearrange("p (g q) -> p g q", q=QW)
                st, sp = (c == 0), (c == DC - 1)
                nc.tensor.matmul(qt_ps, lhsT=wq_sb[:, c, :],
                                 rhs=xr[:, 0:3:2, :], start=st, stop=sp,
                                 skip_group_check=True)
                for i in range(2):
                    nc.tensor.matmul(kt_ps[i], lhsT=wk_sb[:, c, :],
                                     rhs=xh[:, 512 * i:512 * (i + 1)],
                                     start=st, stop=sp, skip_group_check=True)
                for i in range(2):
                    nc.tensor.matmul(vt_ps[i], lhsT=wv_sb[:, c, :],
                                     rhs=xh[:, 512 * i:512 * (i + 1)],
                                     start=st, stop=sp, skip_group_check=True)
            # copies (vt first: transposes wait on them). For half 0,
            # balance DVE/ACT; for half 1 keep ACT free — attA's exps
            # run concurrently with these copies.
            if h == 0:
                nc.vector.tensor_copy(vt_q[0], vt_ps[0])
                nc.scalar.copy(vt_q[1], vt_ps[1])
                nc.vector.tensor_copy(kt_q[0], kt_ps[0])
                nc.scalar.copy(kt_q[1], kt_ps[1])
                nc.vector.tensor_copy(qt_h[h], qt_ps)
            else:
                nc.vector.tensor_copy(vt_q[2], vt_ps[0])
                nc.scalar.copy(vt_q[3], vt_ps[1])
                nc.vector.tensor_copy(kt_q[2], kt_ps[0])
                nc.scalar.copy(kt_q[3], kt_ps[1])
                nc.vector.tensor_copy(qt_h[h], qt_ps)

        def transposes(h, trp):
            for b in range(8):
                ptr = trp.tile([P, P], F16, tag="tr", bufs=2,
                               name=f"ptr{h}_{b}")
                nc.tensor.transpose(
                    ptr, vt_q[2 * h + b // 4][:, P * (b % 4):P * (b % 4 + 1)],
                    ident)
                nc.vector.tensor_copy(v_h[h][:, P * b:P * (b + 1)], ptr)

        def attention_pair(a, att):
            """Pair a: slots {2a, 2a+1}; pair q cols 0..511 = qt halves
            a (lo slot, cols 0..255) and hi slot (256..511)... pair q =
            [slot 2a | slot 2a+1] = qt_h[a] fully. Key groups 0..2a-1
            full, group 2a diag-masked on slot-lo, group 2a+1 slot-hi
            only."""
            jlo, jhi = 2 * a, 2 * a + 1
            nfull = jhi            # groups 0..jhi-1 stream 512q
            qt_pair = qt_h[a]
            qt_hi = qt_h[a][:, 256:512]

            den_ps = att.tile([1, 512], F32, tag="den", bufs=1,
                              name=f"den_ps{a}")
            ctx_ps = att.tile([P, 512], F32, tag="ctx", bufs=1,
                              name=f"ctx_ps{a}")
            pt = {}

            def st_exp(g):
                """Scores (+additive mask on diag regions) + exp for key
                group g -> its own PT tile. PT layouts: wide groups
                [4kb x 512q]; narrow [4kb x 256q]."""
                wide = g < nfull or g == jlo
                qw = 512 if wide else 256
                qtr = qt_pair if wide else qt_hi
                ptile = pt_pool.tile([P, 2048], F16, tag="pt",
                                     name=f"pt{a}_{g}")
                pt[g] = ptile
                diag = (g == jlo) if wide else True
                nhalf = 2 if wide else 1
                for half in range(nhalf):
                    stp = att.tile([P, 1024], F32, tag="st", bufs=2,
                                   name=f"st{a}_{g}_{half}")
                    for k2h in range(4 // nhalf):
                        k2 = half * 2 + k2h if wide else k2h
                        kb = 4 * g + k2
                        base = qw * k2h  # kb's col base in stp
                        nc.tensor.matmul(stp[:, base:base + qw],
                                         lhsT=kt_blk(kb), rhs=qtr,
                                         start=True, stop=not diag,
                                         skip_group_check=True)
                        if not diag:
                            continue
                        # additive causal mask (identity-matmul accum) on
                        # the diag slot's 256 q (= first 256 cols of the
                        # kb when wide). kb1/kb3: q0 sub-block is all
                        # -inf for both parities -> den/av skip it, so
                        # only the q1 half needs the mask rows.
                        if k2 in (0, 2):
                            nc.tensor.matmul(
                                stp[:, base:base + 256],
                                lhsT=ident, rhs=maskt_sb[:, k2, :],
                                start=False, stop=True,
                                skip_group_check=True)
                        else:
                            nc.tensor.matmul(
                                stp[:, base + P:base + 256],
                                lhsT=ident, rhs=maskt_sb[:, k2, P:QW],
                                start=False, stop=True,
                                skip_group_check=True)
                    off = 1024 * half
                    nc.scalar.activation(
                        out=ptile[:, off:off + 1024], in_=stp,
                        func=mybir.ActivationFunctionType.Exp,
                        bias=expbias, scale=SCALE)

            ngroups = jhi + 1
            ndenav = 0

            def denav(g):
                """den+AV for key group g. Skips regions that the causal
                mask provably zeroes for both parities: diag-group kb1/
                kb3 slot-lo halves."""
                nonlocal ndenav
                wide = g < nfull or g == jlo
                ptile = pt[g]
                first = (g == 0)
                last = (ndenav == ngroups - 1)
                parts = []  # (kb, pt_off, den_off, width)
                for k2 in range(4):
                    kb = 4 * g + k2
                    if wide:
                        base = 512 * k2
                        if g == jlo and k2 in (1, 3):
                            # skip slot-lo q0 sub-block (all-masked)
                            parts.append((kb, base + P, P, 512 - P))
                        else:
                            parts.append((kb, base, 0, 512))
                    else:
                        base = 256 * k2
                        if k2 in (1, 3):
                            parts.append((kb, base + P, 256 + P, P))
                        else:
                            parts.append((kb, base, 256, 256))
                for i, (kb, po, do, w) in enumerate(parts):
                    nc.tensor.matmul(den_ps[:, do:do + w], lhsT=ones,
                                     rhs=ptile[:, po:po + w],
                                     start=(first and i == 0),
                                     stop=(last and i == len(parts) - 1),
                                     skip_group_check=True)
                for i, (kb, po, do, w) in enumerate(parts):
                    nc.tensor.matmul(ctx_ps[:, do:do + w], lhsT=v_blk(kb),
                                     rhs=ptile[:, po:po + w],
                                     start=(first and i == 0),
                                     stop=(last and i == len(parts) - 1),
                                     skip_group_check=True)
                ndenav += 1

            # interleave: scores run one group ahead of den/av
            st_exp(0)
            for g in range(1, ngroups):
                st_exp(g)
                denav(g - 1)
            denav(ngroups - 1)

            # finish: ctx/den to SBUF; den partition-ified by PE
            # transpose-matmuls (lhsT = [1,128] den row, rhs = ident[0,0]
            # -> [128,1] column) into a psum tile sharing den's bank;
            # then ONE ACT Reciprocal reads psum and writes rden SBUF.
            # (The earlier gpsimd-DMA broadcast put ~2us of SWDGE latency
            # on the last pair's critical path.)
            nc.vector.tensor_copy(ctxt_p[a], ctx_ps)
            nc.vector.tensor_copy(den_p[a], den_ps)
            dent_ps = att.tile([P, 4], F32, tag="den", bufs=1,
                               name=f"dent_ps{a}")
            for qb2 in range(4):
                nc.tensor.matmul(dent_ps[:, qb2:qb2 + 1],
                                 lhsT=den_p[a][0:1, P * qb2:P * (qb2 + 1)],
                                 rhs=onef32,
                                 start=(qb2 == 0), stop=(qb2 == 3),
                                 skip_group_check=True)
            nc.vector.reciprocal(rden_p[a], dent_ps)

        def out_block(qb, att, act_heavy=False, split_store=False,
                      op_bufs=2):
            """Output projection for 128-q block qb (qt col block). 1/den
            scaling fuses into the mandatory psum->SBUF copies. While ACT
            is exp-bound (pair-A blocks) it takes 1 of 4 halves; after
            the last exp (pair-B blocks) the split is even (act_heavy).
            split_store issues each half's store as soon as it's scaled
            (for the last block, whose store is the kernel tail)."""
            a, qb2 = qb // 4, qb % 4
            rd = rden_p[a][:, qb2:qb2 + 1]
            ctxb = ctxt_p[a][:, P * qb2:P * (qb2 + 1)]
            ot = out_pool.tile([P, D_MODEL], F16, tag="ot")
            for t in range(2):
                ps = att.tile([P, 512], F32, tag="op", bufs=op_bufs,
                              name=f"op{qb}_{t}")
                nc.tensor.matmul(ps, lhsT=ctxb,
                                 rhs=wo_sb[:, 512 * t:512 * (t + 1)],
                                 start=True, stop=True,
                                 skip_group_check=True)
                on_act = (t == 1) if act_heavy else (t == 1 and qb % 2 == 0)
                if on_act:
                    nc.scalar.mul(ot[:, 512 * t:512 * (t + 1)], ps, rd)
                else:
                    nc.vector.tensor_scalar_mul(
                        ot[:, 512 * t:512 * (t + 1)], ps, rd)
                if split_store:
                    nc.sync.dma_start(
                        out=out_d[P * qb:P * (qb + 1),
                                  512 * t:512 * (t + 1)],
                        in_=ot[:, 512 * t:512 * (t + 1)])
            if not split_store:
                nc.sync.dma_start(out=out_d[P * qb:P * (qb + 1), :], in_=ot)

        # ---------------- schedule ----------------
        # PE warm-up: the HAM clock gate needs ~3.4us of sustained PE
        # activity to lift the clock from 1.2 to 2.4 GHz, and the first
        # ~4us of the kernel is DMA-wait. Burn it on dummy matmuls so the
        # projections run warm (measured ~6us of cold-clock otherwise).
        with tc.tile_pool(name="warm", bufs=1, space="PSUM") as warm:
            wps = warm.tile([P, P], F32, name="warm_ps")
            for _ in range(40):
                nc.tensor.matmul(wps, lhsT=ident, rhs=ident,
                                 start=True, stop=True,
                                 skip_group_check=True)
        # Both projections back to back (the whole proj phase is paced by
        # the single x DMA stream; pjB's matmuls fill pjA's arrival gaps).
        # PSUM budget forces the sequencing: pjA(5) -> trA(2)+pjB(5)=7 ->
        # trB(2)+attA(6)=8 -> attB(8).
        with tc.tile_pool(name="pjA", bufs=1, space="PSUM") as pjA:
            proj_half(0, pjA)
        with tc.tile_pool(name="trA", bufs=1, space="PSUM") as trA:
            transposes(0, trA)
            with tc.tile_pool(name="pjB", bufs=1, space="PSUM") as pjB:
                proj_half(1, pjB)
        with tc.tile_pool(name="trB", bufs=1, space="PSUM") as trB:
            transposes(1, trB)
            with tc.tile_pool(name="attA", bufs=1, space="PSUM") as attA:
                attention_pair(0, attA)
        with tc.tile_pool(name="attB", bufs=1, space="PSUM") as attB:
            # pair-A output blocks first: their deps (ctxt/rden) are done
            # by now, so they fill the attA->attB seam. (Moving them
            # after pair B corrupts results — don't.)
            for qb in range(4):
                out_block(qb, attB)
            attention_pair(1, attB)
        with tc.tile_pool(name="attB2", bufs=1, space="PSUM") as attB2:
            # pair-B outs in their own pool: the score banks are dead, so
            # op gets bufs=4 and the scale copies pipeline fully.
            for qb in range(4, 8):
                out_block(qb, attB2, act_heavy=True, split_store=(qb == 7),
                          op_bufs=4)


_NC_CACHE = None


def build_nc() -> bass.Bass:
    global _NC_CACHE
    if _NC_CACHE is not None:
        return _NC_CACHE
    nc = bacc.Bacc("TRN2", target_bir_lowering=False, debug=False)
    xt_d = nc.dram_tensor("xt", [D_MODEL, SEQ], F16, kind="ExternalInput").ap()
    wq_d = nc.dram_tensor("wq", [P, DC * D_HEAD], F16, kind="ExternalInput").ap()
    wk_d = nc.dram_tensor("wk", [P, DC * D_HEAD], F16, kind="ExternalInput").ap()
    wv_d = nc.dram_tensor("wv", [P, DC * D_HEAD], F16, kind="ExternalInput").ap()
    wo_d = nc.dram_tensor("wo", [D_HEAD, D_MODEL], F16, kind="ExternalInput").ap()
    maskt_d = nc.dram_tensor("maskt", [P, 4 * QW], F16, kind="ExternalInput").ap()
    out_d = nc.dram_tensor("out", [NQ, D_MODEL], F16, kind="ExternalOutput").ap()
    with tile.TileContext(nc) as tc:
        _attention_kernel(tc, xt_d, wq_d, wk_d, wv_d, wo_d, maskt_d, out_d)
    nc.compile()
    _NC_CACHE = nc
    return nc


def _chunk_major(w):
    """[1024, 128] -> [128, 8*128]: row p holds chunks c of w[128c+p, :]."""
    return np.ascontiguousarray(
        w.reshape(DC, P, D_HEAD).transpose(1, 0, 2).reshape(P, DC * D_HEAD))


def kernel(x, wq, wk, wv, wo, _trace=False, _trace_kwargs=None):
    x = np.asarray(x, dtype=np.float32)
    wq_h = _chunk_major(np.asarray(wq, dtype=np.float32).astype(np.float16))
    wk_h = _chunk_major(np.asarray(wk, dtype=np.float32).astype(np.float16))
    wv_h = _chunk_major(np.asarray(wv, dtype=np.float32).astype(np.float16))
    wo_h = np.ascontiguousarray(np.asarray(wo, dtype=np.float32).astype(np.float16))

    nc = build_nc()

    masks = {}
    for p in (0, 1):
        m = (1.0 - make_mask01(p).astype(np.float32)) * MASK_NEG  # additive
        m = m.astype(np.float16)  # [512 k, 256 q]
        masks[p] = np.ascontiguousarray(
            m.reshape(4, P, QW).transpose(1, 0, 2).reshape(P, 4 * QW))
    in_maps = []
    for core in range(NCORES):
        b, parity = core // 2, core % 2
        order = block_order(parity)
        perm = np.concatenate([np.arange(P) + P * o for o in order])
        xt = np.ascontiguousarray(x[b][perm, :].T.astype(np.float16))
        in_maps.append({
            "xt": xt, "wq": wq_h, "wk": wk_h, "wv": wv_h, "wo": wo_h,
            "maskt": masks[parity],
        })

    res = run_bass_kernel_spmd(
        nc, in_maps, core_ids=list(range(NCORES)),
        trace=_trace, **(_trace_kwargs or {}))

    out = np.empty_like(x)
    for core in range(NCORES):
        b, parity = core // 2, core % 2
        order = block_order(parity)
        core_out = res.results[core]["out"].astype(np.float32)
        for j in range(NSLOT):
            for i in range(2):
                qb = order[4 * j + i]
                out[b, P * qb:P * (qb + 1), :] = \
                    core_out[QW * j + P * i:QW * j + P * (i + 1), :]
    if _trace:
        return out, res
    return out
